# revision 19
# baseline (speedup 1.0000x reference)
"""GuidedAttentionLoss on 8 Trainium2 NeuronCores — fp8 DoubleRow edition.

Math: loss = mean_b( sum_{f<F_b, l<L_b} A[b,f,l] * w[b,f,l] / F_b ),
      w = 1 - exp(-c*(l/L - f/F)^2),  c = 1/(2*gamma^(2*step)).

Separable identity: exp(-c(x-y)^2) = exp(-cx^2)*exp(-cy^2)*exp(2cxy);
exp(z) on z in [0, 2c] is a degree-D polynomial, so
  e[f,l] = sum_k h_k[f] * g_k[l],
  h_k[f] = a_k * (2c*y)^k * exp(-c*y^2),  y = f/F   (k = 0..D)
  g_k[l] = x^k * exp(-c*x^2),             x = l/L  (host side).
Then sum_{f,l} A*e = sum_k sum_l g_k[l] * C[k,l] with
  C[k,l] = sum_f h_k[f] * A[f,l]  -- a tiny-M matmul streamed on TensorE.
An all-ones column gives sum_f A for the "1" term.

Device dtype: fp8 e4m3 for BOTH operands with perf_mode=DoubleRow
(2 fp8 MACs/cell/cycle, contraction 256 rows per matmul).  A's element
rounding (~3% rel) averages out over the ~1M-element reduction
(measured 1.3e-5 on the final loss).  The weights h are split into
hi + lo e4m3 columns with per-column power-of-2 scales (host dequant),
recovering ~8 mantissa bits.

Layout: host pre-packs each (core, slot) DRAM buffer as [128, W] fp8,
p-major: the slot's h-weight pairs first, then its A data -- one fully
contiguous DMA per slot carries everything that slot's matmuls need
(the last three slots are split in two so the matmuls gated by the
final DMA-completion receipt cover only a third of a slot).  The odd
trailing 128-row block uses a normal-mode fp8 matmul.  C leaves the
device as float16 scaled by 2^-8 (PSUM f32 C reaches ~2e5).

Sharding: pure data parallel over batch: 8 slots x 8 cores (SPMD).
Slot shapes are (max kblocks, max Lpad) over the slot's 8 batches; a
local-search assignment minimizes sum_i kb_i * Lm_i (the per-core HBM
bytes, the roofline term for this memory-bound kernel).  Slots are
processed smallest-last to shorten the post-stream drain.
"""

import numpy as np
import ml_dtypes

import concourse.bass as bass  # noqa: F401
import concourse.tile as tile
from concourse import bacc, mybir
from concourse.bass_utils import run_bass_kernel_spmd

B, T_DEC, T_ENC = 64, 2048, 512
G_STEPS, GAMMA = 20000, 0.99995
N_CORES = 8
SLOTS = B // N_CORES

FP8 = ml_dtypes.float8_e4m3  # TRN float8e4 (max 240, IEEE-style)
OUT_SCALE = 2.0 ** -8        # C output downscale into f16 range


def _fit_exp_poly(zmax: float) -> np.ndarray:
    """Monomial coeffs a_k with exp(z) ~= sum a_k z^k on [0, zmax].

    Tolerance keyed to fp8 operand noise (~3e-3 per element): 1e-4
    relative keeps the poly error below the quantization noise while
    minimizing the degree (and so the on-device weight columns).
    """
    from numpy.polynomial import chebyshev as C

    zs = np.linspace(0.0, zmax, 4001)
    ez = np.exp(zs)
    for deg in range(5, 31):
        a = C.cheb2poly(C.chebfit(zs, ez, deg))
        err = np.max(np.abs(np.polynomial.polynomial.polyval(zs, a) - ez))
        if err < 1e-4 * np.exp(zmax):
            return a
    return a


def _plan(input_lengths: np.ndarray, target_lengths: np.ndarray):
    """Assign 64 batches to 8 slots x 8 cores minimizing per-core bytes.

    Per-core bytes = sum_i 128 * kb_i * Lm_i with kb_i = max ceil(F/128),
    Lm_i = max pad16(L) over the slot.  Seeded by sort heuristics, then
    single-swap local search (exact incremental cost).
    """
    kb = ((target_lengths.astype(np.int64) + 127) // 128).tolist()
    Lp = (((input_lengths.astype(np.int64) + 15) // 16) * 16).tolist()
    kbA = np.array(kb)
    LpA = np.array(Lp)

    def cost(a):
        return int((kbA[a].max(1) * LpA[a].max(1)).sum())

    seeds = [
        np.argsort(-(kbA * LpA), kind="stable"),
        np.lexsort((-LpA, -kbA)),
        np.argsort(-kbA, kind="stable"),
        np.argsort(-LpA, kind="stable"),
    ]
    cands = [np.stack([o[i * N_CORES:(i + 1) * N_CORES]
                       for i in range(SLOTS)]) for o in seeds]
    best = min(cands, key=cost)
    flat = [int(x) for x in best.flatten()]

    def slot_cost(sl):
        return (max(kb[b] for b in sl) * max(Lp[b] for b in sl))

    cur = [slot_cost(flat[i * 8:(i + 1) * 8]) for i in range(SLOTS)]
    rng = np.random.default_rng(12345)
    pairs = rng.integers(0, B, size=(300000, 2))
    for i, j in pairs:
        si, sj = i // 8, j // 8
        if si == sj:
            continue
        flat[i], flat[j] = flat[j], flat[i]
        ci = slot_cost(flat[si * 8:(si + 1) * 8])
        cj = slot_cost(flat[sj * 8:(sj + 1) * 8])
        if ci + cj <= cur[si] + cur[sj]:
            cur[si], cur[sj] = ci, cj
        else:
            flat[i], flat[j] = flat[j], flat[i]

    assign = np.array(flat).reshape(SLOTS, N_CORES)
    slot_batches = [assign[i] for i in range(SLOTS)]
    slot_kb = [int(kbA[s].max()) for s in slot_batches]
    slot_L = [int(LpA[s].max()) for s in slot_batches]

    # Processing order: second-smallest slot first (earliest first
    # matmul), smallest last (shortest post-stream drain tail).
    sizes = [slot_kb[i] * slot_L[i] for i in range(SLOTS)]
    asc = sorted(range(SLOTS), key=lambda i: sizes[i])
    order = [asc[1]] + asc[2:][::-1] + [asc[0]]
    slot_batches = [slot_batches[i] for i in order]
    slot_kb = [slot_kb[i] for i in order]
    slot_L = [slot_L[i] for i in order]
    return slot_batches, slot_kb, slot_L


def _build_program(slot_kb, slot_L, M, Mp):
    f16 = mybir.dt.float16
    fp8 = mybir.dt.float8e4
    pair_cnt = [(kb + 1) // 2 for kb in slot_kb]  # incl. half pair
    DR = mybir.MatmulPerfMode.DoubleRow

    nc = bacc.Bacc(
        "TRN2", target_bir_lowering=False, debug=False, num_devices=N_CORES
    )
    # One DRAM buffer per slot: the slot's h-weight pairs followed by
    # its A data, both p-major — a single fully-contiguous DMA carries
    # everything a slot's matmuls need.
    wid = [pair_cnt[i] * 2 * Mp for i in range(SLOTS)]
    tot = [wid[i] + slot_kb[i] * slot_L[i] for i in range(SLOTS)]
    a_dr = [
        nc.dram_tensor(f"a{i}", [128, tot[i]], fp8, kind="ExternalInput")
        for i in range(SLOTS)
    ]
    c_dr = [
        nc.dram_tensor(f"c{i}", [M, slot_L[i]], f16, kind="ExternalOutput")
        for i in range(SLOTS)
    ]

    with tile.TileContext(nc) as tc:
        with (
            tc.tile_pool(name="ap", bufs=1) as apool,
            tc.tile_pool(name="op", bufs=4) as opool,
            tc.tile_pool(name="pp", bufs=4, space="PSUM") as pspool,
        ):
            ats = [
                apool.tile([128, tot[i]], fp8, tag=f"a{i}", name=f"at{i}")
                for i in range(SLOTS)
            ]
            for i in range(SLOTS):
                if i >= SLOTS - 3:
                    # Tail slots: two pieces, so the matmuls gated by the
                    # final DMA-completion receipt cover only ~1/3 of the
                    # slot.  Split at an even kblock (pairs don't straddle).
                    kbb = max(2, (slot_kb[i] // 3 + 1) // 2 * 2)
                    sp = tot[i] - kbb * slot_L[i]
                    nc.sync.dma_start(ats[i][:, :sp], a_dr[i][:, :sp])
                    nc.sync.dma_start(ats[i][:, sp:], a_dr[i][:, sp:])
                else:
                    nc.sync.dma_start(ats[i][:, :], a_dr[i][:, :])
            for i in range(SLOTS):
                kbn = slot_kb[i]
                Lm = slot_L[i]
                wv = ats[i][:, :wid[i]].rearrange(
                    "p (j two mp) -> p j two mp", two=2, mp=Mp)
                dv = ats[i][:, wid[i]:].rearrange(
                    "p (kb l) -> p kb l", kb=kbn)
                ps = pspool.tile([M, Lm], mybir.dt.float32, tag="ps",
                                 name=f"ps{i}")
                npair = kbn // 2
                half = kbn % 2
                for j in range(npair):
                    nc.tensor.matmul(
                        ps[:, :],
                        wv[:, j, :, :M],
                        dv[:, 2 * j:2 * j + 2, :],
                        start=(j == 0),
                        stop=(j == npair - 1 and not half),
                        perf_mode=DR,
                    )
                if half:
                    nc.tensor.matmul(
                        ps[:, :],
                        wv[:, npair, 0, :M],
                        dv[:, kbn - 1, :],
                        start=(npair == 0),
                        stop=True,
                    )
                ot = opool.tile([M, Lm], mybir.dt.float16, tag="o",
                                name=f"ot{i}")
                # pow2 downscale keeps C (up to ~2e5) inside f16 range;
                # the host epilogue multiplies it back out.
                nc.scalar.mul(ot[:, :], ps[:, :], OUT_SCALE)
                nc.scalar.dma_start(c_dr[i][:, :], ot[:, :])
    nc.compile()
    return nc, wid


def _pow2_scale(maxabs: float, target: float = 160.0) -> float:
    if maxabs <= 0:
        return 1.0
    return float(2.0 ** np.round(np.log2(target / maxabs)))


def _prepare(alignments, input_lengths, target_lengths, global_step):
    """Build program + per-core input maps. Returns (nc, in_maps, meta)."""
    step = int(global_step)
    g = GAMMA ** step
    c = 1.0 / (2.0 * g * g)
    a_poly = _fit_exp_poly(2.0 * c)
    D = len(a_poly) - 1
    M1 = D + 1
    M = 2 * M1 + 1                      # hi cols, lo cols, ones
    Mp = ((2 * M1 + 1 + 15) // 16) * 16  # layout stride (>= M, mult of 16)

    F = target_lengths.astype(np.int64)
    L = input_lengths.astype(np.int64)
    slot_batches, slot_kb, slot_L = _plan(input_lengths, target_lengths)

    nc, wid = _build_program(slot_kb, slot_L, M, Mp)

    al = np.asarray(alignments, dtype=np.float32)
    in_maps = []
    scales = [[None] * SLOTS for _ in range(N_CORES)]
    for j in range(N_CORES):
        im = {}
        for i in range(SLOTS):
            b = int(slot_batches[i][j])
            kbn, Lm = slot_kb[i], slot_L[i]
            R = kbn * 128
            a = al[b, :R, :Lm].astype(FP8)         # [R, Lm]
            a_pack = np.ascontiguousarray(
                a.reshape(kbn, 128, Lm).transpose(1, 0, 2)).reshape(128, -1)
            Fb = int(F[b])
            y = np.arange(R, dtype=np.float64) / Fb
            h = np.zeros((R, M1), dtype=np.float64)
            for k in range(M1):
                h[:, k] = a_poly[k] * (2.0 * c * y) ** k * np.exp(-c * y * y)
            h[Fb:, :] = 0.0
            s = np.empty(M1)
            u = np.empty(M1)
            hq = np.zeros((R, Mp), dtype=FP8)
            for k in range(M1):
                s[k] = _pow2_scale(np.abs(h[:, k]).max())
                hi = (h[:, k] * s[k]).astype(FP8)
                r = h[:, k] * s[k] - hi.astype(np.float64)
                u[k] = _pow2_scale(np.abs(r).max())
                hq[:, k] = hi
                hq[:, M1 + k] = (r * u[k]).astype(FP8)
            ones = np.zeros(R)
            ones[:min(Fb, R)] = 1.0
            hq[:, 2 * M1] = ones.astype(FP8)
            scales[j][i] = (s, u)
            # [R, Mp] -> [128, pairs*2*Mp]: row = blk*128 + p, blk = 2*pr+e
            npr = (kbn + 1) // 2
            hq_pad = np.zeros((npr * 256, Mp), dtype=FP8)
            hq_pad[:R] = hq
            h_pack = np.ascontiguousarray(
                hq_pad.reshape(npr, 2, 128, Mp).transpose(2, 0, 1, 3)
            ).reshape(128, -1)
            assert h_pack.shape[1] == wid[i]
            im[f"a{i}"] = np.concatenate([h_pack, a_pack], axis=1)
        in_maps.append(im)

    meta = dict(slot_batches=slot_batches, slot_kb=slot_kb, slot_L=slot_L,
                scales=scales, M1=M1, c=c, F=F, L=L)
    return nc, in_maps, meta


def _finish(results, meta):
    """Host epilogue: tiny [M, L] combinations per batch, f64."""
    slot_batches = meta["slot_batches"]
    scales = meta["scales"]
    M1, c, F, L = meta["M1"], meta["c"], meta["F"], meta["L"]
    per_sample = np.zeros(B, dtype=np.float64)
    for j in range(N_CORES):
        for i in range(SLOTS):
            b = int(slot_batches[i][j])
            Lb = int(L[b])
            Cm = results[j][f"c{i}"].astype(np.float64) / OUT_SCALE
            s, u = scales[j][i]
            Ck = (Cm[:M1, :Lb] + Cm[M1:2 * M1, :Lb] / u[:, None]) / s[:, None]
            x = np.arange(Lb, dtype=np.float64) / Lb
            ex = np.exp(-c * x * x)
            gsum = np.zeros(Lb)
            xk = np.ones(Lb)
            for k in range(M1):
                gsum += Ck[k] * xk
                xk *= x
            per_sample[b] = Cm[2 * M1, :Lb].sum() - (gsum * ex).sum()
    loss = np.float64(np.mean(per_sample / F.astype(np.float64)))
    return np.asarray(loss, dtype=np.float32)


def _kernel_impl(alignments, input_lengths, target_lengths, global_step,
                 trace=False):
    step = int(global_step)
    if G_STEPS < step:
        return np.zeros((), dtype=np.float32), None
    nc, in_maps, meta = _prepare(alignments, input_lengths, target_lengths,
                                 global_step)
    res = run_bass_kernel_spmd(nc, in_maps, list(range(N_CORES)), trace=trace)
    return _finish(res.results, meta), res


def kernel(alignments, input_lengths, target_lengths, global_step):
    loss, _ = _kernel_impl(alignments, input_lengths, target_lengths,
                           global_step)
    return loss


# revision 21
# speedup vs baseline: 1.1362x; 1.1362x over previous
"""GuidedAttentionLoss on 8 Trainium2 NeuronCores — fp8 DoubleRow edition.

Math: loss = mean_b( sum_{f<F_b, l<L_b} A[b,f,l] * w[b,f,l] / F_b ),
      w = 1 - exp(-c*(l/L - f/F)^2),  c = 1/(2*gamma^(2*step)).

Separable identity: exp(-c(x-y)^2) = exp(-cx^2)*exp(-cy^2)*exp(2cxy);
exp(z) on z in [0, 2c] is a degree-D polynomial, so
  e[f,l] = sum_k h_k[f] * g_k[l],
  h_k[f] = a_k * (2c*y)^k * exp(-c*y^2),  y = f/F   (k = 0..D)
  g_k[l] = x^k * exp(-c*x^2),             x = l/L  (host side).
Then sum_{f,l} A*e = sum_k sum_l g_k[l] * C[k,l] with
  C[k,l] = sum_f h_k[f] * A[f,l]  -- a tiny-M matmul streamed on TensorE.
An all-ones column gives sum_f A for the "1" term.

Device dtype: fp8 e4m3 for BOTH operands with perf_mode=DoubleRow
(2 fp8 MACs/cell/cycle, contraction 256 rows per matmul).  A's element
rounding (~3% rel) averages out over the ~1M-element reduction
(measured 1.3e-5 on the final loss).  The weights h are split into
hi + lo e4m3 columns with per-column power-of-2 scales (host dequant),
recovering ~8 mantissa bits.

Layout: host pre-packs each (core, slot) DRAM buffer as [128, W] fp8,
p-major: the slot's h-weight pairs first, then its A data -- one fully
contiguous DMA per slot carries everything that slot's matmuls need
(the last three slots are split in two so the matmuls gated by the
final DMA-completion receipt cover only a third of a slot).  The odd
trailing 128-row block uses a normal-mode fp8 matmul.  C leaves the
device as float16 scaled by 2^-8 (PSUM f32 C reaches ~2e5).

Sharding: pure data parallel over batch: 8 slots x 8 cores (SPMD).
Slot shapes are (max kblocks, max Lpad) over the slot's 8 batches; a
local-search assignment minimizes sum_i kb_i * Lm_i (the per-core HBM
bytes, the roofline term for this memory-bound kernel).  Slots are
processed smallest-last to shorten the post-stream drain.
"""

import numpy as np
import ml_dtypes

import concourse.bass as bass  # noqa: F401
import concourse.tile as tile
from concourse import bacc, mybir
from concourse.bass_utils import run_bass_kernel_spmd

B, T_DEC, T_ENC = 64, 2048, 512
G_STEPS, GAMMA = 20000, 0.99995
N_CORES = 8
SLOTS = B // N_CORES

FP8 = ml_dtypes.float8_e4m3  # TRN float8e4 (max 240, IEEE-style)
OUT_SCALE = 2.0 ** -8        # C output downscale into f16 range


def _fit_exp_poly(zmax: float) -> np.ndarray:
    """Monomial coeffs a_k with exp(z) ~= sum a_k z^k on [0, zmax].

    Tolerance keyed to fp8 operand noise (~3e-3 per element): 1e-4
    relative keeps the poly error below the quantization noise while
    minimizing the degree (and so the on-device weight columns).
    """
    from numpy.polynomial import chebyshev as C

    zs = np.linspace(0.0, zmax, 4001)
    ez = np.exp(zs)
    for deg in range(5, 31):
        a = C.cheb2poly(C.chebfit(zs, ez, deg))
        err = np.max(np.abs(np.polynomial.polynomial.polyval(zs, a) - ez))
        if err < 1e-4 * np.exp(zmax):
            return a
    return a


def _plan(input_lengths: np.ndarray, target_lengths: np.ndarray):
    """Assign 64 batches to 8 slots x 8 cores minimizing per-core bytes.

    Per-core bytes = sum_i 128 * kb_i * Lm_i with kb_i = max ceil(F/128),
    Lm_i = max pad16(L) over the slot.  Seeded by sort heuristics, then
    single-swap local search (exact incremental cost).
    """
    kb = ((target_lengths.astype(np.int64) + 127) // 128).tolist()
    Lp = (((input_lengths.astype(np.int64) + 15) // 16) * 16).tolist()
    kbA = np.array(kb)
    LpA = np.array(Lp)

    def cost(a):
        return int((kbA[a].max(1) * LpA[a].max(1)).sum())

    seeds = [
        np.argsort(-(kbA * LpA), kind="stable"),
        np.lexsort((-LpA, -kbA)),
        np.argsort(-kbA, kind="stable"),
        np.argsort(-LpA, kind="stable"),
    ]
    cands = [np.stack([o[i * N_CORES:(i + 1) * N_CORES]
                       for i in range(SLOTS)]) for o in seeds]
    best = min(cands, key=cost)
    flat = [int(x) for x in best.flatten()]

    def slot_cost(sl):
        return (max(kb[b] for b in sl) * max(Lp[b] for b in sl))

    cur = [slot_cost(flat[i * 8:(i + 1) * 8]) for i in range(SLOTS)]
    rng = np.random.default_rng(12345)
    pairs = rng.integers(0, B, size=(300000, 2))
    for i, j in pairs:
        si, sj = i // 8, j // 8
        if si == sj:
            continue
        flat[i], flat[j] = flat[j], flat[i]
        ci = slot_cost(flat[si * 8:(si + 1) * 8])
        cj = slot_cost(flat[sj * 8:(sj + 1) * 8])
        if ci + cj <= cur[si] + cur[sj]:
            cur[si], cur[sj] = ci, cj
        else:
            flat[i], flat[j] = flat[j], flat[i]

    assign = np.array(flat).reshape(SLOTS, N_CORES)
    slot_batches = [assign[i] for i in range(SLOTS)]
    slot_kb = [int(kbA[s].max()) for s in slot_batches]
    slot_L = [int(LpA[s].max()) for s in slot_batches]

    # Processing order: second-smallest slot first (earliest first
    # matmul), smallest last (shortest post-stream drain tail).
    sizes = [slot_kb[i] * slot_L[i] for i in range(SLOTS)]
    asc = sorted(range(SLOTS), key=lambda i: sizes[i])
    order = [asc[1]] + asc[2:][::-1] + [asc[0]]
    slot_batches = [slot_batches[i] for i in order]
    slot_kb = [slot_kb[i] for i in order]
    slot_L = [slot_L[i] for i in order]
    return slot_batches, slot_kb, slot_L


def _build_program(slot_kb, slot_L, M, Mp):
    f16 = mybir.dt.float16
    fp8 = mybir.dt.float8e4
    pair_cnt = [(kb + 1) // 2 for kb in slot_kb]  # incl. half pair
    DR = mybir.MatmulPerfMode.DoubleRow

    nc = bacc.Bacc(
        "TRN2", target_bir_lowering=False, debug=False, num_devices=N_CORES
    )
    # One DRAM buffer per slot: the slot's h-weight pairs followed by
    # its A data, both p-major — a single fully-contiguous DMA carries
    # everything a slot's matmuls need.
    wid = [pair_cnt[i] * 2 * Mp for i in range(SLOTS)]
    tot = [wid[i] + slot_kb[i] * slot_L[i] for i in range(SLOTS)]
    a_dr = [
        nc.dram_tensor(f"a{i}", [128, tot[i]], fp8, kind="ExternalInput")
        for i in range(SLOTS)
    ]
    c_dr = [
        nc.dram_tensor(f"c{i}", [M, slot_L[i]], f16, kind="ExternalOutput")
        for i in range(SLOTS)
    ]

    with tile.TileContext(nc) as tc:
        with (
            tc.tile_pool(name="ap", bufs=1) as apool,
            tc.tile_pool(name="op", bufs=4) as opool,
            tc.tile_pool(name="pp", bufs=4, space="PSUM") as pspool,
        ):
            ats = [
                apool.tile([128, tot[i]], fp8, tag=f"a{i}", name=f"at{i}")
                for i in range(SLOTS)
            ]
            for i in range(SLOTS):
                if i >= SLOTS - 3:
                    # Tail slots: two pieces, so the matmuls gated by the
                    # final DMA-completion receipt cover only ~1/3 of the
                    # slot.  First piece ends on an EVEN kblock so no
                    # DoubleRow pair straddles the pieces.
                    kba = (slot_kb[i] - max(2, slot_kb[i] // 3)) // 2 * 2
                    sp = wid[i] + kba * slot_L[i]
                    nc.sync.dma_start(ats[i][:, :sp], a_dr[i][:, :sp])
                    nc.sync.dma_start(ats[i][:, sp:], a_dr[i][:, sp:])
                else:
                    nc.sync.dma_start(ats[i][:, :], a_dr[i][:, :])
            for i in range(SLOTS):
                kbn = slot_kb[i]
                Lm = slot_L[i]
                wv = ats[i][:, :wid[i]].rearrange(
                    "p (j two mp) -> p j two mp", two=2, mp=Mp)
                dv = ats[i][:, wid[i]:].rearrange(
                    "p (kb l) -> p kb l", kb=kbn)
                ps = pspool.tile([M, Lm], mybir.dt.float32, tag="ps",
                                 name=f"ps{i}")
                npair = kbn // 2
                half = kbn % 2
                for j in range(npair):
                    nc.tensor.matmul(
                        ps[:, :],
                        wv[:, j, :, :M],
                        dv[:, 2 * j:2 * j + 2, :],
                        start=(j == 0),
                        stop=(j == npair - 1 and not half),
                        perf_mode=DR,
                    )
                if half:
                    nc.tensor.matmul(
                        ps[:, :],
                        wv[:, npair, 0, :M],
                        dv[:, kbn - 1, :],
                        start=(npair == 0),
                        stop=True,
                    )
                ot = opool.tile([M, Lm], mybir.dt.float16, tag="o",
                                name=f"ot{i}")
                # pow2 downscale keeps C (up to ~2e5) inside f16 range;
                # the host epilogue multiplies it back out.  For the two
                # tail slots the PSUM drain is split across ACT and DVE
                # to halve the copy on the critical drain path.
                if i >= SLOTS - 2:
                    h1 = (Lm // 2 + 7) // 8 * 8
                    nc.scalar.mul(ot[:, :h1], ps[:, :h1], OUT_SCALE)
                    nc.vector.tensor_scalar_mul(
                        ot[:, h1:], ps[:, h1:], OUT_SCALE)
                else:
                    nc.scalar.mul(ot[:, :], ps[:, :], OUT_SCALE)
                nc.scalar.dma_start(c_dr[i][:, :], ot[:, :])
    nc.compile()
    return nc, wid


def _pow2_scale(maxabs: float, target: float = 160.0) -> float:
    if maxabs <= 0:
        return 1.0
    return float(2.0 ** np.round(np.log2(target / maxabs)))


def _prepare(alignments, input_lengths, target_lengths, global_step):
    """Build program + per-core input maps. Returns (nc, in_maps, meta)."""
    step = int(global_step)
    g = GAMMA ** step
    c = 1.0 / (2.0 * g * g)
    a_poly = _fit_exp_poly(2.0 * c)
    D = len(a_poly) - 1
    M1 = D + 1
    M = 2 * M1 + 1                      # hi cols, lo cols, ones
    Mp = ((2 * M1 + 1 + 15) // 16) * 16  # layout stride (>= M, mult of 16)

    F = target_lengths.astype(np.int64)
    L = input_lengths.astype(np.int64)
    slot_batches, slot_kb, slot_L = _plan(input_lengths, target_lengths)

    nc, wid = _build_program(slot_kb, slot_L, M, Mp)

    al = np.asarray(alignments, dtype=np.float32)
    in_maps = []
    scales = [[None] * SLOTS for _ in range(N_CORES)]
    for j in range(N_CORES):
        im = {}
        for i in range(SLOTS):
            b = int(slot_batches[i][j])
            kbn, Lm = slot_kb[i], slot_L[i]
            R = kbn * 128
            a = al[b, :R, :Lm].astype(FP8)         # [R, Lm]
            a_pack = np.ascontiguousarray(
                a.reshape(kbn, 128, Lm).transpose(1, 0, 2)).reshape(128, -1)
            Fb = int(F[b])
            y = np.arange(R, dtype=np.float64) / Fb
            h = np.zeros((R, M1), dtype=np.float64)
            for k in range(M1):
                h[:, k] = a_poly[k] * (2.0 * c * y) ** k * np.exp(-c * y * y)
            h[Fb:, :] = 0.0
            s = np.empty(M1)
            u = np.empty(M1)
            hq = np.zeros((R, Mp), dtype=FP8)
            for k in range(M1):
                s[k] = _pow2_scale(np.abs(h[:, k]).max())
                hi = (h[:, k] * s[k]).astype(FP8)
                r = h[:, k] * s[k] - hi.astype(np.float64)
                u[k] = _pow2_scale(np.abs(r).max())
                hq[:, k] = hi
                hq[:, M1 + k] = (r * u[k]).astype(FP8)
            ones = np.zeros(R)
            ones[:min(Fb, R)] = 1.0
            hq[:, 2 * M1] = ones.astype(FP8)
            scales[j][i] = (s, u)
            # [R, Mp] -> [128, pairs*2*Mp]: row = blk*128 + p, blk = 2*pr+e
            npr = (kbn + 1) // 2
            hq_pad = np.zeros((npr * 256, Mp), dtype=FP8)
            hq_pad[:R] = hq
            h_pack = np.ascontiguousarray(
                hq_pad.reshape(npr, 2, 128, Mp).transpose(2, 0, 1, 3)
            ).reshape(128, -1)
            assert h_pack.shape[1] == wid[i]
            im[f"a{i}"] = np.concatenate([h_pack, a_pack], axis=1)
        in_maps.append(im)

    meta = dict(slot_batches=slot_batches, slot_kb=slot_kb, slot_L=slot_L,
                scales=scales, M1=M1, c=c, F=F, L=L)
    return nc, in_maps, meta


def _finish(results, meta):
    """Host epilogue: tiny [M, L] combinations per batch, f64."""
    slot_batches = meta["slot_batches"]
    scales = meta["scales"]
    M1, c, F, L = meta["M1"], meta["c"], meta["F"], meta["L"]
    per_sample = np.zeros(B, dtype=np.float64)
    for j in range(N_CORES):
        for i in range(SLOTS):
            b = int(slot_batches[i][j])
            Lb = int(L[b])
            Cm = results[j][f"c{i}"].astype(np.float64) / OUT_SCALE
            s, u = scales[j][i]
            Ck = (Cm[:M1, :Lb] + Cm[M1:2 * M1, :Lb] / u[:, None]) / s[:, None]
            x = np.arange(Lb, dtype=np.float64) / Lb
            ex = np.exp(-c * x * x)
            gsum = np.zeros(Lb)
            xk = np.ones(Lb)
            for k in range(M1):
                gsum += Ck[k] * xk
                xk *= x
            per_sample[b] = Cm[2 * M1, :Lb].sum() - (gsum * ex).sum()
    loss = np.float64(np.mean(per_sample / F.astype(np.float64)))
    return np.asarray(loss, dtype=np.float32)


def _kernel_impl(alignments, input_lengths, target_lengths, global_step,
                 trace=False):
    step = int(global_step)
    if G_STEPS < step:
        return np.zeros((), dtype=np.float32), None
    nc, in_maps, meta = _prepare(alignments, input_lengths, target_lengths,
                                 global_step)
    res = run_bass_kernel_spmd(nc, in_maps, list(range(N_CORES)), trace=trace)
    return _finish(res.results, meta), res


def kernel(alignments, input_lengths, target_lengths, global_step):
    loss, _ = _kernel_impl(alignments, input_lengths, target_lengths,
                           global_step)
    return loss
